# revision 5
# baseline (speedup 1.0000x reference)
"""Trainium2 Bass kernel for nn_Geometrical_Pen (segment_reduce, memory-bound).

Computes n_pen[i] = dot(x_normals[i], y_normals[i]) / ||y_normals[0]||
for N = 16,777,216 vertices, D = 3.

Strategy (data-parallel over 8 NeuronCores):
  - Shard both [N,3] inputs along the vertex axis: 2,097,152 vertices/core.
  - Host computes the scalar 1/||y_normals[0]|| (3 floats); it is baked into
    the program as an immediate (the Bass program is built per kernel() call).
  - Per core: stream tiles of 128 partitions x F vertices ([128, 3F] f32
    contiguous HWDGE DMA loads, 3 MiB for F=2048), then on the Vector engine:
      1. tensor_mul: prod = x * y (in place)
      2. tensor_reduce over the innermost D=3 axis (AP [128, F, 3] -> X)
    then scale by 1/||y0|| on the Scalar engine and store from its HWDGE
    ring (decouples store triggers from load triggers on Sync).
  - A tail of small tiles keeps the end-of-pipeline drain short.
  - Memory-bound: 48 MiB in + 8 MiB out per core; measured ~160-165 us/core
    (~143 us pure DMA at line rate + startup/drain/barrier overhead).
"""

import sys

for _p in ("/opt/trn_rl_repo",):
    if _p not in sys.path:
        sys.path.insert(0, _p)

import numpy as np

import concourse.bacc as bacc
import concourse.mybir as mybir
from concourse.bass_utils import run_bass_kernel_spmd
from concourse.tile import TileContext


def _ensure_axon_ntff_hook():
    """Provide antenv.axon_hooks if the image's antenv lacks it.

    concourse.bass_utils unconditionally imports
    antenv.axon_hooks.get_axon_ntff_profile_hook when trace=True under
    axon; on images whose antenv predates that module the import raises
    and kills the run. Register a compatible shim backed by the same
    ctypes calls the axon boot uses, so NTFF profiling works (or
    degrades to a skipped trace when the .so lacks the symbols).
    """
    try:
        import antenv.axon_hooks  # noqa: F401

        return
    except ImportError:
        pass

    import contextlib
    import ctypes
    import types

    def _make_hook():
        so_path = "/opt/axon/libaxon_pjrt.so"
        try:
            lib = ctypes.CDLL(so_path)
        except OSError:
            return None
        if not hasattr(lib, "axon_start_nrt_profile"):
            return None
        lib.axon_start_nrt_profile.argtypes = [
            ctypes.POINTER(ctypes.c_int64),
            ctypes.c_size_t,
        ]
        lib.axon_start_nrt_profile.restype = ctypes.c_int64
        lib.axon_stop_nrt_profile.argtypes = [ctypes.c_char_p]
        lib.axon_stop_nrt_profile.restype = ctypes.c_int64

        @contextlib.contextmanager
        def _hook(output_dir, device_ids):
            import jax

            jax.devices()  # ensure the PJRT client exists in this process
            if device_ids:
                ids = (ctypes.c_int64 * len(device_ids))(*device_ids)
                rc = lib.axon_start_nrt_profile(ids, len(device_ids))
            else:
                rc = lib.axon_start_nrt_profile(None, 0)
            if rc != 0:
                raise RuntimeError(f"axon_start_nrt_profile rc={rc}")
            try:
                yield
            finally:
                n = lib.axon_stop_nrt_profile(str(output_dir).encode())
                if n < 0:
                    raise RuntimeError(f"axon_stop_nrt_profile rc={n}")
                print(f"ntff profile: {n} file(s) written to {output_dir}")

        return _hook

    holder = {"hook": _make_hook()}
    mod = types.ModuleType("antenv.axon_hooks")
    mod.get_axon_ntff_profile_hook = lambda: holder["hook"]

    def _set(h):
        holder["hook"] = h

    mod.set_axon_ntff_profile_hook = _set
    sys.modules["antenv.axon_hooks"] = mod
    try:
        import antenv

        antenv.axon_hooks = mod
    except ImportError:
        pass


_ensure_axon_ntff_hook()

N = 16777216
D = 3
NCORES = 8
P = 128                      # SBUF partitions
SHARD = N // NCORES          # 2,097,152 vertices per core

# Results of the last device run (for test harnesses to read timing info).
LAST_RESULTS = None
_NC_CACHE = {}


# Tile schedule: small tiles at both ends (fast pipeline fill, short
# drain), uniform 1024-column tiles in the middle.
TILE_FS = [512] * 2 + [1024] * 14 + [512] * 2
assert sum(TILE_FS) * P == SHARD

BUFS = 3

# Demand pacing. All 8 cores race their HWDGE load queues at the ~435
# GB/s per-NC DMA-fabric cap, but the chip only has 716 GB/s per HBM
# stack shared by each NC pair (2.86 TB/s total, and the aggregate trace
# shows it fully saturated). The arbitration between stack-mates is
# unfair: the odd physical NC always streams at ~390-435 GB/s while the
# even one is starved down to ~150-300 GB/s in its tail, spreading
# per-core exec from 160 to 194 us (the graded number is the max).
# Fix: cap each core's steady demand at its fair share so no queue ever
# backs up and winners/losers can't form. The pacer is the DVE: dummy
# tensor_reduce ops sized so each tile's DVE chain lasts as long as the
# HBM fair-share time of the tile it gates (pool slot j is recycled by
# tile j+BUFS, so the load of tile j+BUFS issues when tile j's chain
# ends). Elastic by construction: a temporarily-starved core's chain
# just slips later; it never over-demands to catch up.
R_PACE = 358e9       # per-core fair share of stack bandwidth, B/s
ELEM_NS = 1.06       # measured DVE fp32 cost, ns per element per partition
OP_OH_NS = 100.0     # measured per-instruction overhead, ns
BYTES_PER_VERTEX = 28  # 12 x + 12 y in, 4 out


def _dve_real_ns(f: int) -> float:
    # mul over 3f elems + grouped reduce over 3f elems
    return 2.0 * (3 * f * ELEM_NS + OP_OH_NS)


def _period_ns(f: int) -> float:
    return P * f * BYTES_PER_VERTEX / R_PACE * 1e9


def _emit_dummies(nc, scr_ap, xt, yt, f: int, total_ns: float):
    """Emit DVE reduce ops reading yt/xt worth ~total_ns of DVE time.

    Reading the tiles (not scratch) is what extends their pool-slot
    lifetime to the end of the chain, which is the pacing gate."""
    srcs = [yt, xt]
    i = 0
    remaining = total_ns
    while remaining > OP_OH_NS + 64 * ELEM_NS:
        w = min(3 * f, int((remaining - OP_OH_NS) / ELEM_NS))
        src = srcs[i % 2]
        nc.vector.tensor_reduce(
            out=scr_ap,
            in_=src[:, :w],
            axis=mybir.AxisListType.X,
            op=mybir.AluOpType.add,
        )
        remaining -= w * ELEM_NS + OP_OH_NS
        i += 1


def _build_nc(inv_len: float):
    # Bacc (not plain Bass): its compile pipeline legalizes instructions
    # with more than one semaphore wait, which this walrus build rejects.
    nc = bacc.Bacc(None, target_bir_lowering=False)
    x = nc.dram_tensor("x", [SHARD * D], mybir.dt.float32, kind="ExternalInput")
    y = nc.dram_tensor("y", [SHARD * D], mybir.dt.float32, kind="ExternalInput")
    out = nc.dram_tensor("out", [SHARD], mybir.dt.float32, kind="ExternalOutput")

    ntiles = len(TILE_FS)
    scr = nc.alloc_sbuf_tensor("pace_scr", [P, 1], mybir.dt.float32)
    with TileContext(nc) as tc:
        with tc.tile_pool(name="sbuf", bufs=BUFS) as pool:
            v0 = 0  # vertex offset within the shard
            for j, tf in enumerate(TILE_FS):
                vt = P * tf
                xt = pool.tile([P, D * tf], mybir.dt.float32, tag="x")
                yt = pool.tile([P, D * tf], mybir.dt.float32, tag="y")
                st = pool.tile([P, tf], mybir.dt.float32, tag="s")
                xs = x[v0 * D:(v0 + vt) * D].rearrange("(p m) -> p m", p=P)
                ys = y[v0 * D:(v0 + vt) * D].rearrange("(p m) -> p m", p=P)
                nc.sync.dma_start(out=xt[:], in_=xs)
                nc.sync.dma_start(out=yt[:], in_=ys)
                # prod = x * y, in place into the x tile (DVE)
                nc.vector.tensor_mul(out=xt[:], in0=xt[:], in1=yt[:])
                # grouped sum over the innermost D=3 components (DVE)
                nc.vector.tensor_reduce(
                    out=st[:],
                    in_=xt[:].rearrange("p (f d) -> p f d", d=D),
                    axis=mybir.AxisListType.X,
                    op=mybir.AluOpType.add,
                )
                # pacing: stretch this tile's DVE chain to the fair-share
                # HBM time of the tile whose load it gates (j + BUFS)
                if j + BUFS < ntiles:
                    pad = _period_ns(TILE_FS[j + BUFS]) - _dve_real_ns(tf)
                    if pad > 0:
                        _emit_dummies(nc, scr.ap(), xt, yt, tf, pad)
                # scale by 1/||y_0|| on the otherwise-idle Scalar engine,
                # and issue the store from its HWDGE ring too, so store
                # triggers don't serialize behind load triggers on Sync.
                nc.scalar.mul(st[:], st[:], inv_len)
                od = out[v0:v0 + vt].rearrange("(p m) -> p m", p=P)
                nc.scalar.dma_start(out=od, in_=st[:])
                v0 += vt
    nc.finalize()
    return nc


def kernel(x_normals: np.ndarray, y_normals: np.ndarray) -> np.ndarray:
    global LAST_RESULTS

    x = np.ascontiguousarray(np.asarray(x_normals, dtype=np.float32))
    y = np.ascontiguousarray(np.asarray(y_normals, dtype=np.float32))
    assert x.shape == (N, D) and y.shape == (N, D)

    y0 = y[0]
    y_len = np.float32(np.sqrt(np.float32(np.sum(y0 * y0, dtype=np.float32))))
    inv_len = float(np.float32(1.0) / y_len)

    xs = x.reshape(NCORES, SHARD * D)
    ys = y.reshape(NCORES, SHARD * D)

    if inv_len not in _NC_CACHE:
        _NC_CACHE[inv_len] = _build_nc(inv_len)
    nc = _NC_CACHE[inv_len]

    in_maps = [{"x": xs[c], "y": ys[c]} for c in range(NCORES)]
    res = run_bass_kernel_spmd(nc, in_maps, core_ids=list(range(NCORES)))
    LAST_RESULTS = res

    out = np.concatenate([r["out"].reshape(-1) for r in res.results])
    return out



# revision 6
# speedup vs baseline: 1.0351x; 1.0351x over previous
"""Trainium2 Bass kernel for nn_Geometrical_Pen (segment_reduce, memory-bound).

Computes n_pen[i] = dot(x_normals[i], y_normals[i]) / ||y_normals[0]||
for N = 16,777,216 vertices, D = 3.

Strategy (data-parallel over 8 NeuronCores):
  - Shard both [N,3] inputs along the vertex axis: 2,097,152 vertices/core.
  - Host computes the scalar 1/||y_normals[0]|| (3 floats); it is baked into
    the program as an immediate (the Bass program is built per kernel() call).
  - Per core: stream tiles of 128 partitions x F vertices ([128, 3F] f32
    contiguous HWDGE DMA loads, 3 MiB for F=2048), then on the Vector engine:
      1. tensor_mul: prod = x * y (in place)
      2. tensor_reduce over the innermost D=3 axis (AP [128, F, 3] -> X)
    then scale by 1/||y0|| on the Scalar engine and store from its HWDGE
    ring (decouples store triggers from load triggers on Sync).
  - A tail of small tiles keeps the end-of-pipeline drain short.
  - Memory-bound: 48 MiB in + 8 MiB out per core; measured ~160-165 us/core
    (~143 us pure DMA at line rate + startup/drain/barrier overhead).
"""

import sys

for _p in ("/opt/trn_rl_repo",):
    if _p not in sys.path:
        sys.path.insert(0, _p)

import numpy as np

import concourse.bacc as bacc
import concourse.mybir as mybir
from concourse.bass_utils import run_bass_kernel_spmd
from concourse.tile import TileContext


def _ensure_axon_ntff_hook():
    """Provide antenv.axon_hooks if the image's antenv lacks it.

    concourse.bass_utils unconditionally imports
    antenv.axon_hooks.get_axon_ntff_profile_hook when trace=True under
    axon; on images whose antenv predates that module the import raises
    and kills the run. Register a compatible shim backed by the same
    ctypes calls the axon boot uses, so NTFF profiling works (or
    degrades to a skipped trace when the .so lacks the symbols).
    """
    try:
        import antenv.axon_hooks  # noqa: F401

        return
    except ImportError:
        pass

    import contextlib
    import ctypes
    import types

    def _make_hook():
        so_path = "/opt/axon/libaxon_pjrt.so"
        try:
            lib = ctypes.CDLL(so_path)
        except OSError:
            return None
        if not hasattr(lib, "axon_start_nrt_profile"):
            return None
        lib.axon_start_nrt_profile.argtypes = [
            ctypes.POINTER(ctypes.c_int64),
            ctypes.c_size_t,
        ]
        lib.axon_start_nrt_profile.restype = ctypes.c_int64
        lib.axon_stop_nrt_profile.argtypes = [ctypes.c_char_p]
        lib.axon_stop_nrt_profile.restype = ctypes.c_int64

        @contextlib.contextmanager
        def _hook(output_dir, device_ids):
            import jax

            jax.devices()  # ensure the PJRT client exists in this process
            if device_ids:
                ids = (ctypes.c_int64 * len(device_ids))(*device_ids)
                rc = lib.axon_start_nrt_profile(ids, len(device_ids))
            else:
                rc = lib.axon_start_nrt_profile(None, 0)
            if rc != 0:
                raise RuntimeError(f"axon_start_nrt_profile rc={rc}")
            try:
                yield
            finally:
                n = lib.axon_stop_nrt_profile(str(output_dir).encode())
                if n < 0:
                    raise RuntimeError(f"axon_stop_nrt_profile rc={n}")
                print(f"ntff profile: {n} file(s) written to {output_dir}")

        return _hook

    holder = {"hook": _make_hook()}
    mod = types.ModuleType("antenv.axon_hooks")
    mod.get_axon_ntff_profile_hook = lambda: holder["hook"]

    def _set(h):
        holder["hook"] = h

    mod.set_axon_ntff_profile_hook = _set
    sys.modules["antenv.axon_hooks"] = mod
    try:
        import antenv

        antenv.axon_hooks = mod
    except ImportError:
        pass


_ensure_axon_ntff_hook()

N = 16777216
D = 3
NCORES = 8
P = 128                      # SBUF partitions
SHARD = N // NCORES          # 2,097,152 vertices per core

# Results of the last device run (for test harnesses to read timing info).
LAST_RESULTS = None
_NC_CACHE = {}


# Tile schedule: fine-grained uniform tiles. Small tiles mean small DMA
# bursts (~0.77 MB, ~1.8 us at the fabric cap), which is what makes the
# paced demand stream smooth enough that stack-mates' bursts interleave
# instead of phase-locking into winner/loser collisions. A short f=128
# tail shortens the end-of-pipeline DVE backlog.
TILE_FS = [256] * 60 + [128] * 8
assert sum(TILE_FS) * P == SHARD

BUFS = 6

# Demand pacing. All 8 cores race their HWDGE load queues at the ~435
# GB/s per-NC DMA-fabric cap, but the chip only has 716 GB/s per HBM
# stack shared by each NC pair (2.86 TB/s total, and the aggregate trace
# shows it fully saturated). The arbitration between stack-mates is
# unfair: the odd physical NC always streams at ~390-435 GB/s while the
# even one is starved down to ~150-300 GB/s in its tail, spreading
# per-core exec from 160 to 194 us (the graded number is the max).
# Fix: cap each core's steady demand at its fair share so no queue ever
# backs up and winners/losers can't form. The pacer is the DVE: dummy
# tensor_reduce ops sized so each tile's DVE chain lasts as long as the
# HBM fair-share time of the tile it gates (pool slot j is recycled by
# tile j+BUFS, so the load of tile j+BUFS issues when tile j's chain
# ends). Elastic by construction: a temporarily-starved core's chain
# just slips later; it never over-demands to catch up.
R_PACE = 358e9       # per-core fair share of stack bandwidth, B/s
ELEM_NS = 1.06       # measured DVE fp32 cost, ns per element per partition
OP_OH_NS = 100.0     # measured per-instruction overhead, ns
BYTES_PER_VERTEX = 28  # 12 x + 12 y in, 4 out


def _dve_real_ns(f: int) -> float:
    # mul over 3f elems + grouped reduce over 3f elems
    return 2.0 * (3 * f * ELEM_NS + OP_OH_NS)


def _period_ns(f: int) -> float:
    return P * f * BYTES_PER_VERTEX / R_PACE * 1e9


def _emit_dummies(nc, scr_ap, xt, yt, f: int, total_ns: float):
    """Emit DVE reduce ops reading yt/xt worth ~total_ns of DVE time.

    Reading the tiles (not scratch) is what extends their pool-slot
    lifetime to the end of the chain, which is the pacing gate."""
    srcs = [yt, xt]
    i = 0
    remaining = total_ns
    while remaining > OP_OH_NS + 64 * ELEM_NS:
        w = min(3 * f, int((remaining - OP_OH_NS) / ELEM_NS))
        src = srcs[i % 2]
        nc.vector.tensor_reduce(
            out=scr_ap,
            in_=src[:, :w],
            axis=mybir.AxisListType.X,
            op=mybir.AluOpType.add,
        )
        remaining -= w * ELEM_NS + OP_OH_NS
        i += 1


def _build_nc(inv_len: float):
    # Bacc (not plain Bass): its compile pipeline legalizes instructions
    # with more than one semaphore wait, which this walrus build rejects.
    nc = bacc.Bacc(None, target_bir_lowering=False)
    x = nc.dram_tensor("x", [SHARD * D], mybir.dt.float32, kind="ExternalInput")
    y = nc.dram_tensor("y", [SHARD * D], mybir.dt.float32, kind="ExternalInput")
    out = nc.dram_tensor("out", [SHARD], mybir.dt.float32, kind="ExternalOutput")

    ntiles = len(TILE_FS)
    scr = nc.alloc_sbuf_tensor("pace_scr", [P, 1], mybir.dt.float32)
    with TileContext(nc) as tc:
        with tc.tile_pool(name="sbuf", bufs=BUFS) as pool:
            v0 = 0  # vertex offset within the shard
            for j, tf in enumerate(TILE_FS):
                vt = P * tf
                xt = pool.tile([P, D * tf], mybir.dt.float32, tag="x")
                yt = pool.tile([P, D * tf], mybir.dt.float32, tag="y")
                st = pool.tile([P, tf], mybir.dt.float32, tag="s")
                xs = x[v0 * D:(v0 + vt) * D].rearrange("(p m) -> p m", p=P)
                ys = y[v0 * D:(v0 + vt) * D].rearrange("(p m) -> p m", p=P)
                nc.sync.dma_start(out=xt[:], in_=xs)
                nc.sync.dma_start(out=yt[:], in_=ys)
                # prod = x * y, in place into the x tile (DVE)
                nc.vector.tensor_mul(out=xt[:], in0=xt[:], in1=yt[:])
                # grouped sum over the innermost D=3 components (DVE)
                nc.vector.tensor_reduce(
                    out=st[:],
                    in_=xt[:].rearrange("p (f d) -> p f d", d=D),
                    axis=mybir.AxisListType.X,
                    op=mybir.AluOpType.add,
                )
                # pacing: stretch this tile's DVE chain to the fair-share
                # HBM time of the tile whose load it gates (j + BUFS)
                if j + BUFS < ntiles:
                    pad = _period_ns(TILE_FS[j + BUFS]) - _dve_real_ns(tf)
                    if pad > 0:
                        _emit_dummies(nc, scr.ap(), xt, yt, tf, pad)
                # scale by 1/||y_0|| on the otherwise-idle Scalar engine,
                # and issue the store from its HWDGE ring too, so store
                # triggers don't serialize behind load triggers on Sync.
                nc.scalar.mul(st[:], st[:], inv_len)
                od = out[v0:v0 + vt].rearrange("(p m) -> p m", p=P)
                nc.scalar.dma_start(out=od, in_=st[:])
                v0 += vt
    nc.finalize()
    return nc


def kernel(x_normals: np.ndarray, y_normals: np.ndarray) -> np.ndarray:
    global LAST_RESULTS

    x = np.ascontiguousarray(np.asarray(x_normals, dtype=np.float32))
    y = np.ascontiguousarray(np.asarray(y_normals, dtype=np.float32))
    assert x.shape == (N, D) and y.shape == (N, D)

    y0 = y[0]
    y_len = np.float32(np.sqrt(np.float32(np.sum(y0 * y0, dtype=np.float32))))
    inv_len = float(np.float32(1.0) / y_len)

    xs = x.reshape(NCORES, SHARD * D)
    ys = y.reshape(NCORES, SHARD * D)

    if inv_len not in _NC_CACHE:
        _NC_CACHE[inv_len] = _build_nc(inv_len)
    nc = _NC_CACHE[inv_len]

    in_maps = [{"x": xs[c], "y": ys[c]} for c in range(NCORES)]
    res = run_bass_kernel_spmd(nc, in_maps, core_ids=list(range(NCORES)))
    LAST_RESULTS = res

    out = np.concatenate([r["out"].reshape(-1) for r in res.results])
    return out



# revision 9
# speedup vs baseline: 1.0651x; 1.0290x over previous
"""Trainium2 Bass kernel for nn_Geometrical_Pen (segment_reduce, memory-bound).

Computes n_pen[i] = dot(x_normals[i], y_normals[i]) / ||y_normals[0]||
for N = 16,777,216 vertices, D = 3.

Strategy (data-parallel over 8 NeuronCores):
  - Shard both [N,3] inputs along the vertex axis: 2,097,152 vertices/core.
  - Host computes the scalar 1/||y_normals[0]|| (3 floats); it is baked into
    the program as an immediate (the Bass program is built per kernel() call).
  - Per core: stream tiles of 128 partitions x F vertices ([128, 3F] f32
    contiguous HWDGE DMA loads, 3 MiB for F=2048), then on the Vector engine:
      1. tensor_mul: prod = x * y (in place)
      2. tensor_reduce over the innermost D=3 axis (AP [128, F, 3] -> X)
    then scale by 1/||y0|| on the Scalar engine and store from its HWDGE
    ring (decouples store triggers from load triggers on Sync).
  - A tail of small tiles keeps the end-of-pipeline drain short.
  - Memory-bound: 48 MiB in + 8 MiB out per core; measured ~160-165 us/core
    (~143 us pure DMA at line rate + startup/drain/barrier overhead).
"""

import sys

for _p in ("/opt/trn_rl_repo",):
    if _p not in sys.path:
        sys.path.insert(0, _p)

import numpy as np

import concourse.bacc as bacc
import concourse.mybir as mybir
from concourse.bass_utils import run_bass_kernel_spmd
from concourse.tile import TileContext


def _ensure_axon_ntff_hook():
    """Provide antenv.axon_hooks if the image's antenv lacks it.

    concourse.bass_utils unconditionally imports
    antenv.axon_hooks.get_axon_ntff_profile_hook when trace=True under
    axon; on images whose antenv predates that module the import raises
    and kills the run. Register a compatible shim backed by the same
    ctypes calls the axon boot uses, so NTFF profiling works (or
    degrades to a skipped trace when the .so lacks the symbols).
    """
    try:
        import antenv.axon_hooks  # noqa: F401

        return
    except ImportError:
        pass

    import contextlib
    import ctypes
    import types

    def _make_hook():
        so_path = "/opt/axon/libaxon_pjrt.so"
        try:
            lib = ctypes.CDLL(so_path)
        except OSError:
            return None
        if not hasattr(lib, "axon_start_nrt_profile"):
            return None
        lib.axon_start_nrt_profile.argtypes = [
            ctypes.POINTER(ctypes.c_int64),
            ctypes.c_size_t,
        ]
        lib.axon_start_nrt_profile.restype = ctypes.c_int64
        lib.axon_stop_nrt_profile.argtypes = [ctypes.c_char_p]
        lib.axon_stop_nrt_profile.restype = ctypes.c_int64

        @contextlib.contextmanager
        def _hook(output_dir, device_ids):
            import jax

            jax.devices()  # ensure the PJRT client exists in this process
            if device_ids:
                ids = (ctypes.c_int64 * len(device_ids))(*device_ids)
                rc = lib.axon_start_nrt_profile(ids, len(device_ids))
            else:
                rc = lib.axon_start_nrt_profile(None, 0)
            if rc != 0:
                raise RuntimeError(f"axon_start_nrt_profile rc={rc}")
            try:
                yield
            finally:
                n = lib.axon_stop_nrt_profile(str(output_dir).encode())
                if n < 0:
                    raise RuntimeError(f"axon_stop_nrt_profile rc={n}")
                print(f"ntff profile: {n} file(s) written to {output_dir}")

        return _hook

    holder = {"hook": _make_hook()}
    mod = types.ModuleType("antenv.axon_hooks")
    mod.get_axon_ntff_profile_hook = lambda: holder["hook"]

    def _set(h):
        holder["hook"] = h

    mod.set_axon_ntff_profile_hook = _set
    sys.modules["antenv.axon_hooks"] = mod
    try:
        import antenv

        antenv.axon_hooks = mod
    except ImportError:
        pass


_ensure_axon_ntff_hook()

N = 16777216
D = 3
NCORES = 8
P = 128                      # SBUF partitions
SHARD = N // NCORES          # 2,097,152 vertices per core

# Results of the last device run (for test harnesses to read timing info).
LAST_RESULTS = None
_NC_CACHE = {}


# Tile schedule: small tiles at both ends (fast pipeline fill, short
# drain), uniform 1024-column tiles in the middle.
TILE_FS = [512] * 2 + [1024] * 14 + [512] * 2
assert sum(TILE_FS) * P == SHARD

BUFS = 3
# Pace only the first PACE_TILES tiles; the rest sprint unpaced. Pacing
# the whole run equalizes all cores but removes the end-of-run windfall
# (early finishers vacating bandwidth) that bounds the slowest core's
# tail; pacing only the front keeps cores together early without
# stretching every core's demand to the very end.
PACE_TILES = 10

# Demand pacing. All 8 cores race their HWDGE load queues at the ~435
# GB/s per-NC DMA-fabric cap, but the chip only has 716 GB/s per HBM
# stack shared by each NC pair (2.86 TB/s total, and the aggregate trace
# shows it fully saturated). The arbitration between stack-mates is
# unfair: the odd physical NC always streams at ~390-435 GB/s while the
# even one is starved down to ~150-300 GB/s in its tail, spreading
# per-core exec from 160 to 194 us (the graded number is the max).
# Fix: cap each core's steady demand at its fair share so no queue ever
# backs up and winners/losers can't form. The pacer is the DVE: dummy
# tensor_reduce ops sized so each tile's DVE chain lasts as long as the
# HBM fair-share time of the tile it gates (pool slot j is recycled by
# tile j+BUFS, so the load of tile j+BUFS issues when tile j's chain
# ends). Elastic by construction: a temporarily-starved core's chain
# just slips later; it never over-demands to catch up.
R_PACE = 370e9       # slightly above fair share: bounded racing, elastic
ELEM_NS = 1.06       # measured DVE fp32 cost, ns per element per partition
OP_OH_NS = 100.0     # measured per-instruction overhead, ns
BYTES_PER_VERTEX = 28  # 12 x + 12 y in, 4 out


def _dve_real_ns(f: int) -> float:
    # mul over 3f elems + grouped reduce over 3f elems
    return 2.0 * (3 * f * ELEM_NS + OP_OH_NS)


def _period_ns(f: int) -> float:
    return P * f * BYTES_PER_VERTEX / R_PACE * 1e9


def _emit_dummies(nc, scr_ap, xt, yt, f: int, total_ns: float):
    """Emit DVE reduce ops reading yt/xt worth ~total_ns of DVE time.

    Reading the tiles (not scratch) is what extends their pool-slot
    lifetime to the end of the chain, which is the pacing gate."""
    srcs = [yt, xt]
    i = 0
    remaining = total_ns
    while remaining > OP_OH_NS + 64 * ELEM_NS:
        w = min(3 * f, int((remaining - OP_OH_NS) / ELEM_NS))
        src = srcs[i % 2]
        nc.vector.tensor_reduce(
            out=scr_ap,
            in_=src[:, :w],
            axis=mybir.AxisListType.X,
            op=mybir.AluOpType.add,
        )
        remaining -= w * ELEM_NS + OP_OH_NS
        i += 1


def _build_nc(inv_len: float):
    # Bacc (not plain Bass): its compile pipeline legalizes instructions
    # with more than one semaphore wait, which this walrus build rejects.
    nc = bacc.Bacc(None, target_bir_lowering=False)
    x = nc.dram_tensor("x", [SHARD * D], mybir.dt.float32, kind="ExternalInput")
    y = nc.dram_tensor("y", [SHARD * D], mybir.dt.float32, kind="ExternalInput")
    out = nc.dram_tensor("out", [SHARD], mybir.dt.float32, kind="ExternalOutput")

    ntiles = len(TILE_FS)
    scr = nc.alloc_sbuf_tensor("pace_scr", [P, 1], mybir.dt.float32)
    with TileContext(nc) as tc:
        with tc.tile_pool(name="sbuf", bufs=BUFS) as pool:
            v0 = 0  # vertex offset within the shard
            for j, tf in enumerate(TILE_FS):
                vt = P * tf
                xt = pool.tile([P, D * tf], mybir.dt.float32, tag="x")
                yt = pool.tile([P, D * tf], mybir.dt.float32, tag="y")
                st = pool.tile([P, tf], mybir.dt.float32, tag="s")
                xs = x[v0 * D:(v0 + vt) * D].rearrange("(p m) -> p m", p=P)
                ys = y[v0 * D:(v0 + vt) * D].rearrange("(p m) -> p m", p=P)
                nc.sync.dma_start(out=xt[:], in_=xs)
                nc.sync.dma_start(out=yt[:], in_=ys)
                # prod = x * y, in place into the x tile (DVE)
                nc.vector.tensor_mul(out=xt[:], in0=xt[:], in1=yt[:])
                # grouped sum over the innermost D=3 components (DVE)
                nc.vector.tensor_reduce(
                    out=st[:],
                    in_=xt[:].rearrange("p (f d) -> p f d", d=D),
                    axis=mybir.AxisListType.X,
                    op=mybir.AluOpType.add,
                )
                # pacing: stretch this tile's DVE chain to the fair-share
                # HBM time of the tile whose load it gates (j + BUFS)
                if j + BUFS < ntiles and j < PACE_TILES:
                    pad = _period_ns(TILE_FS[j + BUFS]) - _dve_real_ns(tf)
                    if pad > 0:
                        _emit_dummies(nc, scr.ap(), xt, yt, tf, pad)
                # scale by 1/||y_0|| on the otherwise-idle Scalar engine,
                # and issue the store from its HWDGE ring too, so store
                # triggers don't serialize behind load triggers on Sync.
                nc.scalar.mul(st[:], st[:], inv_len)
                od = out[v0:v0 + vt].rearrange("(p m) -> p m", p=P)
                nc.scalar.dma_start(out=od, in_=st[:])
                v0 += vt
    nc.finalize()
    return nc


def kernel(x_normals: np.ndarray, y_normals: np.ndarray) -> np.ndarray:
    global LAST_RESULTS

    x = np.ascontiguousarray(np.asarray(x_normals, dtype=np.float32))
    y = np.ascontiguousarray(np.asarray(y_normals, dtype=np.float32))
    assert x.shape == (N, D) and y.shape == (N, D)

    y0 = y[0]
    y_len = np.float32(np.sqrt(np.float32(np.sum(y0 * y0, dtype=np.float32))))
    inv_len = float(np.float32(1.0) / y_len)

    xs = x.reshape(NCORES, SHARD * D)
    ys = y.reshape(NCORES, SHARD * D)

    if inv_len not in _NC_CACHE:
        _NC_CACHE[inv_len] = _build_nc(inv_len)
    nc = _NC_CACHE[inv_len]

    in_maps = [{"x": xs[c], "y": ys[c]} for c in range(NCORES)]
    res = run_bass_kernel_spmd(nc, in_maps, core_ids=list(range(NCORES)))
    LAST_RESULTS = res

    out = np.concatenate([r["out"].reshape(-1) for r in res.results])
    return out

